# revision 31
# baseline (speedup 1.0000x reference)
"""Routed per-behavior FFN (MoE-style) Trainium2 kernel.

Reference semantics: for each token t with b = b_seq[t]:
  b == 0      -> output 0
  b in 1..4   -> LN(elu(x W1_b^T + b1_b) W2_b^T + b2_b) * gamma_b + beta_b

Strategy: host sorts tokens by branch and splits them evenly over the 8
cores (metadata-only routing); each core runs a dense grouped FFN over its
compacted token slab in fp32. ELU is composed as relu(x) - relu(1 - exp(x));
its "-1" term is folded into an effective output bias b2 - sum_f W2[:, f],
applied with a K=1 ones matmul. LayerNorm uses bn_stats/bn_aggr with the
(y - mean) * rstd fusion in one tensor_scalar op.
"""

import json

import numpy as np

B, T = 32, 2048
D_MODEL = 256
D_FF = 1024
N_B = 4
NCORES = 8
LN_EPS = 1e-12
NTOK = B * T

# ---------------------------------------------------------------------------
# walrus workaround: this container's compiler accepts at most one sync wait
# per CTRL-class instruction; split extras onto NoOp carriers.
# ---------------------------------------------------------------------------


def _split_excess_waits(bir: dict, max_waits: int = 1) -> None:
    for fn in bir.get("functions", []):
        for blk in fn.get("blocks", []):
            insts = blk.get("instructions")
            if not insts:
                continue
            new = []
            for inst in insts:
                si = inst.get("sync_info")
                waits = (si or {}).get("on_wait") or []
                if len(waits) > max_waits:
                    excess, keep = waits[:-max_waits], waits[-max_waits:]
                    for k, w in enumerate(excess):
                        new.append(
                            {
                                "debug": inst.get("debug", 0),
                                "engine": inst["engine"],
                                "ins": [],
                                "name": f"{inst['name']}-wsplit{k}",
                                "opcode": "NoOp",
                                "outs": [],
                                "sync_info": {"on_update": [], "on_wait": [w]},
                            }
                        )
                    si["on_wait"] = keep
                new.append(inst)
            blk["instructions"] = new


_bir_fix_installed = False


def _install_bir_fix():
    global _bir_fix_installed
    if _bir_fix_installed:
        return
    import concourse.bass_utils as bass_utils
    import concourse.bass2jax as bass2jax

    orig = bass_utils.compile_bir_kernel

    import os as _os

    if _os.environ.get("LDW_OPT"):
        _orig_bvo = bass_utils.bir_verify_and_optimise

        def _bvo(tmpdir, inp="bir.json", outp="file.neff", arch=None, **kw):
            import unittest.mock as _mock

            real_run = bass_utils.run_command

            def run2(argv, **kwargs):
                argv = [
                    a.replace("--enable-ldw-opt=false", "--enable-ldw-opt=true")
                    for a in argv
                ]
                return real_run(argv, **kwargs)

            with _mock.patch.object(bass_utils, "run_command", run2):
                return _orig_bvo(tmpdir, inp, outp, arch, **kw)

        bass_utils.bir_verify_and_optimise = _bvo

    def patched(bir_json, tmpdir, neff_name="file.neff"):
        bir = json.loads(bir_json)
        _split_excess_waits(bir)
        return orig(json.dumps(bir).encode(), tmpdir, neff_name)

    bass_utils.compile_bir_kernel = patched
    bass2jax.compile_bir_kernel = patched

    # Synthesize antenv.axon_hooks (absent in this image) so trace=True can
    # reach the terminal's NTFF profiler via the axon .so.
    import sys
    import types

    if "antenv.axon_hooks" not in sys.modules:
        try:
            from trn_agent_boot.trn_boot import _ntff_profile_via_ctypes

            hook = _ntff_profile_via_ctypes("/opt/axon/libaxon_pjrt.so")
            mod = types.ModuleType("antenv.axon_hooks")
            mod.get_axon_ntff_profile_hook = lambda: hook
            mod.set_axon_ntff_profile_hook = lambda h: None
            sys.modules["antenv.axon_hooks"] = mod
        except Exception:
            pass
    _bir_fix_installed = True


# ---------------------------------------------------------------------------
# device kernel builder
# ---------------------------------------------------------------------------

_BUILD_CACHE = {}


def _chunks(cap, w=512):
    out = []
    off = 0
    while off < cap:
        out.append((off, min(w, cap - off)))
        off += w
    return out


def _build(caps, b1_nonzero, b2_nonzero, gb_nontrivial):
    import os

    mm_dtype = os.environ.get("MM_DTYPE", "f32r")
    key = (tuple(caps), b1_nonzero, b2_nonzero, gb_nontrivial, mm_dtype)
    if key in _BUILD_CACHE:
        return _BUILD_CACHE[key]

    import concourse.bass as bass
    import concourse.tile as tile
    from concourse import mybir

    f32 = mybir.dt.float32

    fmm = mybir.dt.float32r if mm_dtype == "f32r" else f32
    S = sum(caps)
    KD = D_MODEL // 128  # 2 chunks of the model dim
    KF = D_FF // 128  # 8 chunks of the ff dim

    nc = bass.Bass("TRN2")
    xg = nc.dram_tensor("xg", [KD, 128, S], fmm, kind="ExternalInput")
    w1t = nc.dram_tensor("w1t", [N_B, KD, 128, D_FF], fmm, kind="ExternalInput")
    D2 = D_MODEL + 2
    w2t = nc.dram_tensor("w2t", [N_B, KF, 128, D2], fmm, kind="ExternalInput")
    if b2_nonzero:
        b2e = nc.dram_tensor("b2e", [N_B, D2], f32, kind="ExternalInput")
    if b1_nonzero:
        b1d = nc.dram_tensor("b1", [N_B, D_FF], f32, kind="ExternalInput")
    if gb_nontrivial:
        gamd = nc.dram_tensor("gamma", [N_B, D_MODEL], f32, kind="ExternalInput")
        betd = nc.dram_tensor("beta", [N_B, D_MODEL], f32, kind="ExternalInput")
    yc = nc.dram_tensor("yc", [S, D_MODEL], f32, kind="ExternalOutput")

    AF = mybir.ActivationFunctionType
    OP = mybir.AluOpType

    with tile.TileContext(nc) as tc:
        with (
            tc.tile_pool(name="singles", bufs=1) as singles,
            tc.tile_pool(name="w1p", bufs=2) as w1p,
            tc.tile_pool(name="w2p", bufs=2) as w2p,
            tc.tile_pool(name="cns", bufs=2) as cns,
            tc.tile_pool(name="xp", bufs=3) as xp,
            tc.tile_pool(name="hp", bufs=2) as hp,
            tc.tile_pool(name="ep", bufs=3) as ep,
            tc.tile_pool(name="up", bufs=3) as up,
            tc.tile_pool(name="op_", bufs=8) as op_,
            tc.tile_pool(name="stp", bufs=8) as stp,
            tc.tile_pool(name="php", bufs=3, space="PSUM") as php,
            tc.tile_pool(name="pyp", bufs=2, space="PSUM") as pyp,
        ):
            ones_col = singles.tile([1, 128], f32)
            nc.vector.memset(ones_col, 1.0)
            eps_tile = singles.tile([128, 1], f32)
            nc.vector.memset(eps_tile, LN_EPS)
            if b1_nonzero:
                ones_row = singles.tile([1, 512], f32)
                nc.vector.memset(ones_row, 1.0)

            seg_off = 0
            for n in range(N_B):
                cap = caps[n]
                if cap == 0:
                    continue
                w1_sb = w1p.tile([128, KD, D_FF], fmm, tag="w1")
                for k in range(KD):
                    nc.sync.dma_start(out=w1_sb[:, k, :], in_=w1t[n, k])
                w2_sb = w2p.tile([128, KF, D2], fmm, tag="w2")
                if b2_nonzero:
                    b2e_sb = cns.tile([1, D2], f32, tag="b2e")
                if b1_nonzero:
                    b1_sb = cns.tile([1, D_FF], f32, tag="b1")
                    nc.sync.dma_start(out=b1_sb, in_=b1d[n : n + 1, :])
                if gb_nontrivial:
                    gam_bc = cns.tile([128, D_MODEL], f32, tag="gam")
                    bet_bc = cns.tile([128, D_MODEL], f32, tag="bet")
                    gsrc = gamd[n : n + 1, :]
                    bsrc = betd[n : n + 1, :]
                    nc.gpsimd.dma_start(
                        out=gam_bc,
                        in_=bass.AP(
                            tensor=gsrc.tensor,
                            offset=gsrc.offset,
                            ap=[[0, 128], gsrc.ap[1]],
                        ),
                    )
                    nc.gpsimd.dma_start(
                        out=bet_bc,
                        in_=bass.AP(
                            tensor=bsrc.tensor,
                            offset=bsrc.offset,
                            ap=[[0, 128], bsrc.ap[1]],
                        ),
                    )

                NT = cap // 128
                varall = stp.tile([128, NT], f32, tag="mv")
                rst = stp.tile([128, NT], f32, tag="rst")
                ycs = []
                for off, W in _chunks(cap):
                    goff = seg_off + off
                    nW = W // 128
                    xg_sb = xp.tile([128, KD, 512], fmm, tag="xg")
                    nc.sync.dma_start(
                        out=xg_sb[:, :, :W],
                        in_=xg[:, :, goff : goff + W].rearrange(
                            "k p w -> p k w"
                        ),
                    )
                    if off == 0:
                        nc.sync.dma_start(
                            out=w2_sb, in_=w2t[n].rearrange("j p d -> p j d")
                        )
                        if b2_nonzero:
                            nc.sync.dma_start(out=b2e_sb, in_=b2e[n : n + 1, :])
                    h_sb = hp.tile([128, KF, 512], fmm, tag="h")
                    for fp in range(KF // 2):
                        ph = php.tile([128, 2, 512], f32, tag="ph")
                        for fi in range(2):
                            f = fp * 2 + fi
                            fs = slice(f * 128, (f + 1) * 128)
                            nc.tensor.matmul(
                                ph[:, fi, :W],
                                w1_sb[:, 0, fs],
                                xg_sb[:, 0, :W],
                                start=True,
                                stop=False,
                            )
                            nc.tensor.matmul(
                                ph[:, fi, :W],
                                w1_sb[:, 1, fs],
                                xg_sb[:, 1, :W],
                                start=False,
                                stop=not b1_nonzero,
                            )
                            if b1_nonzero:
                                nc.tensor.matmul(
                                    ph[:, fi, :W],
                                    b1_sb[:, fs],
                                    ones_row[:, :W],
                                    start=False,
                                    stop=True,
                                )
                        # elu(v) = relu(v) - relu(1 - exp(v))
                        e_sb = ep.tile([128, 2, 512], f32, tag="e")
                        nc.scalar.activation(e_sb[:, :, :W], ph[:, :, :W], AF.Exp)
                        u_sb = up.tile([128, 2, 512], f32, tag="u")
                        if fp < 4:
                            # u = relu(1 - E) on ACT (same table set as exp)
                            nc.scalar.activation(
                                u_sb[:, :, :W],
                                e_sb[:, :, :W],
                                AF.Relu,
                                bias=1.0,
                                scale=-1.0,
                            )
                            op1 = OP.subtract
                        else:
                            # v = (E min 1) - 1 = -u on DVE; STT adds it
                            nc.vector.tensor_scalar(
                                u_sb[:, :, :W],
                                e_sb[:, :, :W],
                                scalar1=1.0,
                                scalar2=1.0,
                                op0=OP.min,
                                op1=OP.subtract,
                            )
                            op1 = OP.add
                        nc.vector.scalar_tensor_tensor(
                            h_sb[:, fp * 2 : fp * 2 + 2, :W],
                            ph[:, :, :W],
                            0.0,
                            u_sb[:, :, :W],
                            op0=OP.max,
                            op1=op1,
                        )
                    yc_sb = op_.tile([128, 4, D_MODEL], f32, tag="o")
                    ycs.append((yc_sb, goff, W))
                    for t in range(nW):
                        tt = off // 128 + t
                        py = pyp.tile([128, D2], f32, tag="py")
                        for f in range(KF):
                            nc.tensor.matmul(
                                py,
                                h_sb[:, f, t * 128 : (t + 1) * 128],
                                w2_sb[:, f, :],
                                start=(f == 0),
                                stop=not b2_nonzero and f == KF - 1,
                            )
                        if b2_nonzero:
                            nc.tensor.matmul(
                                py,
                                ones_col[:, :128],
                                b2e_sb,
                                start=False,
                                stop=True,
                            )
                        # py[:, 256] = mean(y) (colsum(W2)/256 folded into w2t).
                        # centered y; scale by rstd later (batched per branch)
                        nc.vector.tensor_scalar_sub(
                            yc_sb[:, t, :],
                            py[:, :D_MODEL],
                            scalar1=py[:, D_MODEL : D_MODEL + 1],
                        )
                        # var = mean((y-mu)^2) via squared sum accumulator
                        vscr = stp.tile([128, D_MODEL], f32, tag="vscr")
                        nc.vector.scalar_tensor_tensor(
                            vscr,
                            yc_sb[:, t, :],
                            0.0,
                            yc_sb[:, t, :],
                            op0=OP.bypass,
                            op1=OP.mult,
                            accum_out=varall[:, tt : tt + 1],
                        )
                # rstd + apply + store. Batched per branch (one sqrt table
                # swap); for the last branch, per chunk-group so the tail
                # pipeline drains during compute.
                def flush(tiles, lo, hi):
                    sd = stp.tile([128, NT], f32, tag="sd")
                    nc.scalar.activation(
                        sd[:, : hi - lo],
                        varall[:, lo:hi],
                        AF.Sqrt,
                        bias=eps_tile,
                        scale=1.0 / D_MODEL,
                    )
                    nc.vector.reciprocal(rst[:, lo:hi], sd[:, : hi - lo])
                    for yc_sb, goff, W in tiles:
                        nW = W // 128
                        o2 = op_.tile([128, 4, D_MODEL], f32, tag="o2")
                        for t in range(nW):
                            tt = (goff - seg_off) // 128 + t
                            nc.vector.tensor_scalar_mul(
                                o2[:, t, :],
                                yc_sb[:, t, :],
                                scalar1=rst[:, tt : tt + 1],
                            )
                            if gb_nontrivial:
                                nc.vector.tensor_mul(
                                    o2[:, t, :], o2[:, t, :], gam_bc
                                )
                                nc.vector.tensor_add(
                                    o2[:, t, :], o2[:, t, :], bet_bc
                                )
                        nc.sync.dma_start(
                            out=yc[goff : goff + W, :].rearrange(
                                "(c p) d -> p c d", p=128
                            ),
                            in_=o2[:, :nW, :],
                        )

                last = n == max(i for i in range(N_B) if caps[i] > 0)
                if last:
                    half = (len(ycs) + 1) // 2
                    for grp in (ycs[:half], ycs[half:]):
                        if not grp:
                            continue
                        lo = (grp[0][1] - seg_off) // 128
                        hi = (grp[-1][1] - seg_off) // 128 + grp[-1][2] // 128
                        flush(grp, lo, hi)
                else:
                    flush(ycs, 0, NT)
                seg_off += cap

    _BUILD_CACHE[key] = nc
    return nc


# ---------------------------------------------------------------------------
# host wrapper
# ---------------------------------------------------------------------------


def kernel(x, b_seq, w1, b1, w2, b2, gamma, beta):
    _install_bir_fix()
    from concourse.bass_utils import run_bass_kernel_spmd

    x = np.asarray(x, dtype=np.float32)
    b_seq = np.asarray(b_seq, dtype=np.int32)
    w1 = np.asarray(w1, dtype=np.float32)
    b1 = np.asarray(b1, dtype=np.float32)
    w2 = np.asarray(w2, dtype=np.float32)
    b2 = np.asarray(b2, dtype=np.float32)
    gamma = np.asarray(gamma, dtype=np.float32)
    beta = np.asarray(beta, dtype=np.float32)

    x_flat = x.reshape(NTOK, D_MODEL)
    bs = b_seq.reshape(NTOK)

    # token ids per branch, split evenly over cores
    parts = []  # parts[n][c] -> int array of token ids
    for n in range(1, N_B + 1):
        idx = np.nonzero(bs == n)[0].astype(np.int64)
        parts.append(np.array_split(idx, NCORES))
    caps = []
    for n in range(N_B):
        mx = max(len(p) for p in parts[n])
        caps.append(0 if mx == 0 else ((mx + 127) // 128) * 128)
    S = sum(caps)

    b1_nonzero = bool(np.any(b1))
    b2_nonzero = bool(np.any(b2))
    gb_nontrivial = bool(np.any(beta)) or not bool(np.all(gamma == 1.0))

    nc = _build(tuple(caps), b1_nonzero, b2_nonzero, gb_nontrivial)

    # weight layouts
    w1t = np.ascontiguousarray(
        w1.transpose(0, 2, 1).reshape(N_B, D_MODEL // 128, 128, D_FF)
    )
    w2t_core = w2.transpose(0, 2, 1).reshape(N_B, D_FF // 128, 128, D_MODEL)
    w2_colmean = (w2.sum(axis=1) / D_MODEL).reshape(N_B, D_FF // 128, 128, 1)
    w2t = np.ascontiguousarray(
        np.concatenate(
            [w2t_core, w2_colmean, np.zeros_like(w2_colmean)], axis=3
        ),
        dtype=np.float32,
    )
    b2e = np.ascontiguousarray(
        np.concatenate(
            [b2, b2.sum(axis=1, keepdims=True) / D_MODEL, np.zeros((N_B, 1), np.float32)],
            axis=1,
        ),
        dtype=np.float32,
    )

    in_maps = []
    gidx_per_core = []
    for c in range(NCORES):
        gidx = np.zeros(S, dtype=np.int64)
        seg = 0
        for n in range(N_B):
            p = parts[n][c]
            gidx[seg : seg + len(p)] = p
            seg += caps[n]
        gidx_per_core.append(gidx)
        xgc = np.ascontiguousarray(x_flat[gidx].T.reshape(D_MODEL // 128, 128, S))
        m = {"xg": xgc, "w1t": w1t, "w2t": w2t}
        if b2_nonzero:
            m["b2e"] = b2e
        if b1_nonzero:
            m["b1"] = b1
        if gb_nontrivial:
            m["gamma"] = gamma
            m["beta"] = beta
        in_maps.append(m)

    import os
    import time

    trace = bool(os.environ.get("KERNEL_TRACE"))
    res = None
    for attempt in range(3):
        try:
            res = run_bass_kernel_spmd(
                nc, in_maps, core_ids=list(range(NCORES)), trace=trace
            )
            break
        except Exception:
            # transient NRT device errors have been observed on the first
            # execution of a freshly compiled NEFF; retry
            if attempt == 2:
                raise
            time.sleep(3)
    global LAST_RESULTS
    LAST_RESULTS = res

    out_flat = np.zeros((NTOK, D_MODEL), dtype=np.float32)
    for c in range(NCORES):
        ycc = res.results[c]["yc"]
        seg = 0
        for n in range(N_B):
            p = parts[n][c]
            out_flat[p] = ycc[seg : seg + len(p)]
            seg += caps[n]
    return out_flat.reshape(B, T, D_MODEL)


# revision 32
# speedup vs baseline: 1.0146x; 1.0146x over previous
"""Routed per-behavior FFN (MoE-style) Trainium2 kernel.

Reference semantics: for each token t with b = b_seq[t]:
  b == 0      -> output 0
  b in 1..4   -> LN(elu(x W1_b^T + b1_b) W2_b^T + b2_b) * gamma_b + beta_b

Strategy: host sorts tokens by branch and splits them evenly over the 8
cores (metadata-only routing); each core runs a dense grouped FFN over its
compacted token slab in fp32. ELU is composed as relu(x) - relu(1 - exp(x));
its "-1" term is folded into an effective output bias b2 - sum_f W2[:, f],
applied with a K=1 ones matmul. LayerNorm uses bn_stats/bn_aggr with the
(y - mean) * rstd fusion in one tensor_scalar op.
"""

import json

import numpy as np

B, T = 32, 2048
D_MODEL = 256
D_FF = 1024
N_B = 4
NCORES = 8
LN_EPS = 1e-12
NTOK = B * T

# ---------------------------------------------------------------------------
# walrus workaround: this container's compiler accepts at most one sync wait
# per CTRL-class instruction; split extras onto NoOp carriers.
# ---------------------------------------------------------------------------


def _split_excess_waits(bir: dict, max_waits: int = 1) -> None:
    for fn in bir.get("functions", []):
        for blk in fn.get("blocks", []):
            insts = blk.get("instructions")
            if not insts:
                continue
            new = []
            for inst in insts:
                si = inst.get("sync_info")
                waits = (si or {}).get("on_wait") or []
                if len(waits) > max_waits:
                    excess, keep = waits[:-max_waits], waits[-max_waits:]
                    for k, w in enumerate(excess):
                        new.append(
                            {
                                "debug": inst.get("debug", 0),
                                "engine": inst["engine"],
                                "ins": [],
                                "name": f"{inst['name']}-wsplit{k}",
                                "opcode": "NoOp",
                                "outs": [],
                                "sync_info": {"on_update": [], "on_wait": [w]},
                            }
                        )
                    si["on_wait"] = keep
                new.append(inst)
            blk["instructions"] = new


_bir_fix_installed = False


def _install_bir_fix():
    global _bir_fix_installed
    if _bir_fix_installed:
        return
    import concourse.bass_utils as bass_utils
    import concourse.bass2jax as bass2jax

    orig = bass_utils.compile_bir_kernel

    import os as _os

    if _os.environ.get("LDW_OPT"):
        _orig_bvo = bass_utils.bir_verify_and_optimise

        def _bvo(tmpdir, inp="bir.json", outp="file.neff", arch=None, **kw):
            import unittest.mock as _mock

            real_run = bass_utils.run_command

            def run2(argv, **kwargs):
                argv = [
                    a.replace("--enable-ldw-opt=false", "--enable-ldw-opt=true")
                    for a in argv
                ]
                return real_run(argv, **kwargs)

            with _mock.patch.object(bass_utils, "run_command", run2):
                return _orig_bvo(tmpdir, inp, outp, arch, **kw)

        bass_utils.bir_verify_and_optimise = _bvo

    def patched(bir_json, tmpdir, neff_name="file.neff"):
        bir = json.loads(bir_json)
        _split_excess_waits(bir)
        return orig(json.dumps(bir).encode(), tmpdir, neff_name)

    bass_utils.compile_bir_kernel = patched
    bass2jax.compile_bir_kernel = patched

    # Synthesize antenv.axon_hooks (absent in this image) so trace=True can
    # reach the terminal's NTFF profiler via the axon .so.
    import sys
    import types

    if "antenv.axon_hooks" not in sys.modules:
        try:
            from trn_agent_boot.trn_boot import _ntff_profile_via_ctypes

            hook = _ntff_profile_via_ctypes("/opt/axon/libaxon_pjrt.so")
            mod = types.ModuleType("antenv.axon_hooks")
            mod.get_axon_ntff_profile_hook = lambda: hook
            mod.set_axon_ntff_profile_hook = lambda h: None
            sys.modules["antenv.axon_hooks"] = mod
        except Exception:
            pass
    _bir_fix_installed = True


# ---------------------------------------------------------------------------
# device kernel builder
# ---------------------------------------------------------------------------

_BUILD_CACHE = {}


def _chunks(cap, w=512):
    out = []
    off = 0
    while off < cap:
        out.append((off, min(w, cap - off)))
        off += w
    return out


def _build(caps, b1_nonzero, b2_nonzero, gb_nontrivial):
    import os

    mm_dtype = os.environ.get("MM_DTYPE", "f32r")
    key = (tuple(caps), b1_nonzero, b2_nonzero, gb_nontrivial, mm_dtype)
    if key in _BUILD_CACHE:
        return _BUILD_CACHE[key]

    import concourse.bass as bass
    import concourse.tile as tile
    from concourse import mybir

    f32 = mybir.dt.float32

    fmm = mybir.dt.float32r if mm_dtype == "f32r" else f32
    S = sum(caps)
    KD = D_MODEL // 128  # 2 chunks of the model dim
    KF = D_FF // 128  # 8 chunks of the ff dim

    nc = bass.Bass("TRN2")
    xg = nc.dram_tensor("xg", [KD, 128, S], fmm, kind="ExternalInput")
    w1t = nc.dram_tensor("w1t", [N_B, KD, 128, D_FF], fmm, kind="ExternalInput")
    D2 = D_MODEL + 2
    w2t = nc.dram_tensor("w2t", [N_B, KF, 128, D2], fmm, kind="ExternalInput")
    if b2_nonzero:
        b2e = nc.dram_tensor("b2e", [N_B, D2], f32, kind="ExternalInput")
    if b1_nonzero:
        b1d = nc.dram_tensor("b1", [N_B, D_FF], f32, kind="ExternalInput")
    if gb_nontrivial:
        gamd = nc.dram_tensor("gamma", [N_B, D_MODEL], f32, kind="ExternalInput")
        betd = nc.dram_tensor("beta", [N_B, D_MODEL], f32, kind="ExternalInput")
    yc = nc.dram_tensor("yc", [S, D_MODEL], f32, kind="ExternalOutput")

    AF = mybir.ActivationFunctionType
    OP = mybir.AluOpType

    with tile.TileContext(nc) as tc:
        with (
            tc.tile_pool(name="singles", bufs=1) as singles,
            tc.tile_pool(name="w1p", bufs=2) as w1p,
            tc.tile_pool(name="w2p", bufs=2) as w2p,
            tc.tile_pool(name="cns", bufs=2) as cns,
            tc.tile_pool(name="xp", bufs=3) as xp,
            tc.tile_pool(name="hp", bufs=2) as hp,
            tc.tile_pool(name="ep", bufs=3) as ep,
            tc.tile_pool(name="up", bufs=3) as up,
            tc.tile_pool(name="op_", bufs=8) as op_,
            tc.tile_pool(name="stp", bufs=8) as stp,
            tc.tile_pool(name="php", bufs=3, space="PSUM") as php,
            tc.tile_pool(name="pyp", bufs=2, space="PSUM") as pyp,
        ):
            ones_col = singles.tile([1, 128], f32)
            nc.vector.memset(ones_col, 1.0)
            eps_tile = singles.tile([128, 1], f32)
            nc.vector.memset(eps_tile, LN_EPS)
            if b1_nonzero:
                ones_row = singles.tile([1, 512], f32)
                nc.vector.memset(ones_row, 1.0)

            seg_off = 0
            for n in range(N_B):
                cap = caps[n]
                if cap == 0:
                    continue
                w1_sb = w1p.tile([128, KD, D_FF], fmm, tag="w1")
                for k in range(KD):
                    nc.sync.dma_start(out=w1_sb[:, k, :], in_=w1t[n, k])
                w2_sb = w2p.tile([128, KF, D2], fmm, tag="w2")
                if b2_nonzero:
                    b2e_sb = cns.tile([1, D2], f32, tag="b2e")
                if b1_nonzero:
                    b1_sb = cns.tile([1, D_FF], f32, tag="b1")
                    nc.sync.dma_start(out=b1_sb, in_=b1d[n : n + 1, :])
                if gb_nontrivial:
                    gam_bc = cns.tile([128, D_MODEL], f32, tag="gam")
                    bet_bc = cns.tile([128, D_MODEL], f32, tag="bet")
                    gsrc = gamd[n : n + 1, :]
                    bsrc = betd[n : n + 1, :]
                    nc.gpsimd.dma_start(
                        out=gam_bc,
                        in_=bass.AP(
                            tensor=gsrc.tensor,
                            offset=gsrc.offset,
                            ap=[[0, 128], gsrc.ap[1]],
                        ),
                    )
                    nc.gpsimd.dma_start(
                        out=bet_bc,
                        in_=bass.AP(
                            tensor=bsrc.tensor,
                            offset=bsrc.offset,
                            ap=[[0, 128], bsrc.ap[1]],
                        ),
                    )

                NT = cap // 128
                varall = stp.tile([128, NT], f32, tag="mv")
                rst = stp.tile([128, NT], f32, tag="rst")
                ycs = []
                for off, W in _chunks(cap):
                    goff = seg_off + off
                    nW = W // 128
                    xg_sb = xp.tile([128, KD, 512], fmm, tag="xg")
                    for k in range(KD):
                        nc.sync.dma_start(
                            out=xg_sb[:, k, :W], in_=xg[k, :, goff : goff + W]
                        )
                    if off == 0:
                        nc.sync.dma_start(
                            out=w2_sb, in_=w2t[n].rearrange("j p d -> p j d")
                        )
                        if b2_nonzero:
                            nc.sync.dma_start(out=b2e_sb, in_=b2e[n : n + 1, :])
                    h_sb = hp.tile([128, KF, 512], fmm, tag="h")
                    for fp in range(KF // 2):
                        ph = php.tile([128, 2, 512], f32, tag="ph")
                        for fi in range(2):
                            f = fp * 2 + fi
                            fs = slice(f * 128, (f + 1) * 128)
                            nc.tensor.matmul(
                                ph[:, fi, :W],
                                w1_sb[:, 0, fs],
                                xg_sb[:, 0, :W],
                                start=True,
                                stop=False,
                            )
                            nc.tensor.matmul(
                                ph[:, fi, :W],
                                w1_sb[:, 1, fs],
                                xg_sb[:, 1, :W],
                                start=False,
                                stop=not b1_nonzero,
                            )
                            if b1_nonzero:
                                nc.tensor.matmul(
                                    ph[:, fi, :W],
                                    b1_sb[:, fs],
                                    ones_row[:, :W],
                                    start=False,
                                    stop=True,
                                )
                        # elu(v) = relu(v) - relu(1 - exp(v))
                        e_sb = ep.tile([128, 2, 512], f32, tag="e")
                        nc.scalar.activation(e_sb[:, :, :W], ph[:, :, :W], AF.Exp)
                        u_sb = up.tile([128, 2, 512], f32, tag="u")
                        if fp < 4:
                            # u = relu(1 - E) on ACT (same table set as exp)
                            nc.scalar.activation(
                                u_sb[:, :, :W],
                                e_sb[:, :, :W],
                                AF.Relu,
                                bias=1.0,
                                scale=-1.0,
                            )
                            op1 = OP.subtract
                        else:
                            # v = (E min 1) - 1 = -u on DVE; STT adds it
                            nc.vector.tensor_scalar(
                                u_sb[:, :, :W],
                                e_sb[:, :, :W],
                                scalar1=1.0,
                                scalar2=1.0,
                                op0=OP.min,
                                op1=OP.subtract,
                            )
                            op1 = OP.add
                        nc.vector.scalar_tensor_tensor(
                            h_sb[:, fp * 2 : fp * 2 + 2, :W],
                            ph[:, :, :W],
                            0.0,
                            u_sb[:, :, :W],
                            op0=OP.max,
                            op1=op1,
                        )
                    yc_sb = op_.tile([128, 4, D_MODEL], f32, tag="o")
                    ycs.append((yc_sb, goff, W))
                    for t in range(nW):
                        tt = off // 128 + t
                        py = pyp.tile([128, D2], f32, tag="py")
                        for f in range(KF):
                            nc.tensor.matmul(
                                py,
                                h_sb[:, f, t * 128 : (t + 1) * 128],
                                w2_sb[:, f, :],
                                start=(f == 0),
                                stop=not b2_nonzero and f == KF - 1,
                            )
                        if b2_nonzero:
                            nc.tensor.matmul(
                                py,
                                ones_col[:, :128],
                                b2e_sb,
                                start=False,
                                stop=True,
                            )
                        # py[:, 256] = mean(y) (colsum(W2)/256 folded into w2t).
                        # centered y; scale by rstd later (batched per branch)
                        nc.vector.tensor_scalar_sub(
                            yc_sb[:, t, :],
                            py[:, :D_MODEL],
                            scalar1=py[:, D_MODEL : D_MODEL + 1],
                        )
                        # var = mean((y-mu)^2) via squared sum accumulator
                        vscr = stp.tile([128, D_MODEL], f32, tag="vscr")
                        nc.vector.scalar_tensor_tensor(
                            vscr,
                            yc_sb[:, t, :],
                            0.0,
                            yc_sb[:, t, :],
                            op0=OP.bypass,
                            op1=OP.mult,
                            accum_out=varall[:, tt : tt + 1],
                        )
                # rstd + apply + store. Batched per branch (one sqrt table
                # swap); for the last branch, per chunk-group so the tail
                # pipeline drains during compute.
                def flush(tiles, lo, hi):
                    sd = stp.tile([128, NT], f32, tag="sd")
                    nc.scalar.activation(
                        sd[:, : hi - lo],
                        varall[:, lo:hi],
                        AF.Sqrt,
                        bias=eps_tile,
                        scale=1.0 / D_MODEL,
                    )
                    nc.vector.reciprocal(rst[:, lo:hi], sd[:, : hi - lo])
                    for yc_sb, goff, W in tiles:
                        nW = W // 128
                        o2 = op_.tile([128, 4, D_MODEL], f32, tag="o2")
                        for t in range(nW):
                            tt = (goff - seg_off) // 128 + t
                            nc.vector.tensor_scalar_mul(
                                o2[:, t, :],
                                yc_sb[:, t, :],
                                scalar1=rst[:, tt : tt + 1],
                            )
                            if gb_nontrivial:
                                nc.vector.tensor_mul(
                                    o2[:, t, :], o2[:, t, :], gam_bc
                                )
                                nc.vector.tensor_add(
                                    o2[:, t, :], o2[:, t, :], bet_bc
                                )
                        nc.sync.dma_start(
                            out=yc[goff : goff + W, :].rearrange(
                                "(c p) d -> p c d", p=128
                            ),
                            in_=o2[:, :nW, :],
                        )

                last = n == max(i for i in range(N_B) if caps[i] > 0)
                if last:
                    half = (len(ycs) + 1) // 2
                    for grp in (ycs[:half], ycs[half:]):
                        if not grp:
                            continue
                        lo = (grp[0][1] - seg_off) // 128
                        hi = (grp[-1][1] - seg_off) // 128 + grp[-1][2] // 128
                        flush(grp, lo, hi)
                else:
                    lowprio = tc.high_priority(offset=-1000000)
                    lowprio.__enter__()
                    flush(ycs, 0, NT)
                    lowprio.__exit__(None, None, None)
                seg_off += cap

    _BUILD_CACHE[key] = nc
    return nc


# ---------------------------------------------------------------------------
# host wrapper
# ---------------------------------------------------------------------------


def kernel(x, b_seq, w1, b1, w2, b2, gamma, beta):
    _install_bir_fix()
    from concourse.bass_utils import run_bass_kernel_spmd

    x = np.asarray(x, dtype=np.float32)
    b_seq = np.asarray(b_seq, dtype=np.int32)
    w1 = np.asarray(w1, dtype=np.float32)
    b1 = np.asarray(b1, dtype=np.float32)
    w2 = np.asarray(w2, dtype=np.float32)
    b2 = np.asarray(b2, dtype=np.float32)
    gamma = np.asarray(gamma, dtype=np.float32)
    beta = np.asarray(beta, dtype=np.float32)

    x_flat = x.reshape(NTOK, D_MODEL)
    bs = b_seq.reshape(NTOK)

    # token ids per branch, split evenly over cores
    parts = []  # parts[n][c] -> int array of token ids
    for n in range(1, N_B + 1):
        idx = np.nonzero(bs == n)[0].astype(np.int64)
        parts.append(np.array_split(idx, NCORES))
    caps = []
    for n in range(N_B):
        mx = max(len(p) for p in parts[n])
        caps.append(0 if mx == 0 else ((mx + 127) // 128) * 128)
    S = sum(caps)

    b1_nonzero = bool(np.any(b1))
    b2_nonzero = bool(np.any(b2))
    gb_nontrivial = bool(np.any(beta)) or not bool(np.all(gamma == 1.0))

    nc = _build(tuple(caps), b1_nonzero, b2_nonzero, gb_nontrivial)

    # weight layouts
    w1t = np.ascontiguousarray(
        w1.transpose(0, 2, 1).reshape(N_B, D_MODEL // 128, 128, D_FF)
    )
    w2t_core = w2.transpose(0, 2, 1).reshape(N_B, D_FF // 128, 128, D_MODEL)
    w2_colmean = (w2.sum(axis=1) / D_MODEL).reshape(N_B, D_FF // 128, 128, 1)
    w2t = np.ascontiguousarray(
        np.concatenate(
            [w2t_core, w2_colmean, np.zeros_like(w2_colmean)], axis=3
        ),
        dtype=np.float32,
    )
    b2e = np.ascontiguousarray(
        np.concatenate(
            [b2, b2.sum(axis=1, keepdims=True) / D_MODEL, np.zeros((N_B, 1), np.float32)],
            axis=1,
        ),
        dtype=np.float32,
    )

    in_maps = []
    gidx_per_core = []
    for c in range(NCORES):
        gidx = np.zeros(S, dtype=np.int64)
        seg = 0
        for n in range(N_B):
            p = parts[n][c]
            gidx[seg : seg + len(p)] = p
            seg += caps[n]
        gidx_per_core.append(gidx)
        xgc = np.ascontiguousarray(x_flat[gidx].T.reshape(D_MODEL // 128, 128, S))
        m = {"xg": xgc, "w1t": w1t, "w2t": w2t}
        if b2_nonzero:
            m["b2e"] = b2e
        if b1_nonzero:
            m["b1"] = b1
        if gb_nontrivial:
            m["gamma"] = gamma
            m["beta"] = beta
        in_maps.append(m)

    import os
    import time

    trace = bool(os.environ.get("KERNEL_TRACE"))
    res = None
    for attempt in range(3):
        try:
            res = run_bass_kernel_spmd(
                nc, in_maps, core_ids=list(range(NCORES)), trace=trace
            )
            break
        except Exception:
            # transient NRT device errors have been observed on the first
            # execution of a freshly compiled NEFF; retry
            if attempt == 2:
                raise
            time.sleep(3)
    global LAST_RESULTS
    LAST_RESULTS = res

    out_flat = np.zeros((NTOK, D_MODEL), dtype=np.float32)
    for c in range(NCORES):
        ycc = res.results[c]["yc"]
        seg = 0
        for n in range(N_B):
            p = parts[n][c]
            out_flat[p] = ycc[seg : seg + len(p)]
            seg += caps[n]
    return out_flat.reshape(B, T, D_MODEL)


# revision 33
# speedup vs baseline: 1.0301x; 1.0153x over previous
"""Routed per-behavior FFN (MoE-style) Trainium2 kernel.

Reference semantics: for each token t with b = b_seq[t]:
  b == 0      -> output 0
  b in 1..4   -> LN(elu(x W1_b^T + b1_b) W2_b^T + b2_b) * gamma_b + beta_b

Strategy:
- Host routing (metadata only): tokens are sorted by branch and each
  branch's token list is split evenly over the 8 cores, so every core runs
  an identical-shape grouped FFN over ~1/8 of the routed tokens (~5x less
  matmul work than computing all 4 branches densely). Gather/transpose of
  x and the final scatter are host-side shard/unshard steps; all FLOPs run
  on device.
- Matmuls run in float32r (single-pass relaxed fp32; plain fp32 matmul
  costs two hardware passes), weight-stationary for W1 and
  activation-stationary for W2 so the LayerNorm reduction axis lands on
  the free dimension.
- ELU is composed exactly as relu(v) - relu(1 - exp(v)): Exp and the
  clamp run on ScalarE (same activation-table set, so no table swaps) and
  one scalar_tensor_tensor on VectorE combines them from PSUM.
- LayerNorm: the mean arrives free as a 257th output column of the W2
  matmul (host appends colsum(W2)/256 to the weights); variance comes from
  one squared-sum accum_out; rstd (sqrt+reciprocal) is batched per branch
  so the sqrt table swap happens 4x per kernel, not per tile.
"""

import json

import numpy as np

B, T = 32, 2048
D_MODEL = 256
D_FF = 1024
N_B = 4
NCORES = 8
LN_EPS = 1e-12
NTOK = B * T

# ---------------------------------------------------------------------------
# walrus workaround: this container's compiler accepts at most one sync wait
# per CTRL-class instruction; split extras onto NoOp carriers.
# ---------------------------------------------------------------------------


def _split_excess_waits(bir: dict, max_waits: int = 1) -> None:
    for fn in bir.get("functions", []):
        for blk in fn.get("blocks", []):
            insts = blk.get("instructions")
            if not insts:
                continue
            new = []
            for inst in insts:
                si = inst.get("sync_info")
                waits = (si or {}).get("on_wait") or []
                if len(waits) > max_waits:
                    excess, keep = waits[:-max_waits], waits[-max_waits:]
                    for k, w in enumerate(excess):
                        new.append(
                            {
                                "debug": inst.get("debug", 0),
                                "engine": inst["engine"],
                                "ins": [],
                                "name": f"{inst['name']}-wsplit{k}",
                                "opcode": "NoOp",
                                "outs": [],
                                "sync_info": {"on_update": [], "on_wait": [w]},
                            }
                        )
                    si["on_wait"] = keep
                new.append(inst)
            blk["instructions"] = new


_bir_fix_installed = False


def _install_bir_fix():
    global _bir_fix_installed
    if _bir_fix_installed:
        return
    import concourse.bass_utils as bass_utils
    import concourse.bass2jax as bass2jax

    orig = bass_utils.compile_bir_kernel

    import os as _os

    if _os.environ.get("LDW_OPT"):
        _orig_bvo = bass_utils.bir_verify_and_optimise

        def _bvo(tmpdir, inp="bir.json", outp="file.neff", arch=None, **kw):
            import unittest.mock as _mock

            real_run = bass_utils.run_command

            def run2(argv, **kwargs):
                argv = [
                    a.replace("--enable-ldw-opt=false", "--enable-ldw-opt=true")
                    for a in argv
                ]
                return real_run(argv, **kwargs)

            with _mock.patch.object(bass_utils, "run_command", run2):
                return _orig_bvo(tmpdir, inp, outp, arch, **kw)

        bass_utils.bir_verify_and_optimise = _bvo

    def patched(bir_json, tmpdir, neff_name="file.neff"):
        bir = json.loads(bir_json)
        _split_excess_waits(bir)
        return orig(json.dumps(bir).encode(), tmpdir, neff_name)

    bass_utils.compile_bir_kernel = patched
    bass2jax.compile_bir_kernel = patched

    # Synthesize antenv.axon_hooks (absent in this image) so trace=True can
    # reach the terminal's NTFF profiler via the axon .so.
    import sys
    import types

    if "antenv.axon_hooks" not in sys.modules:
        try:
            from trn_agent_boot.trn_boot import _ntff_profile_via_ctypes

            hook = _ntff_profile_via_ctypes("/opt/axon/libaxon_pjrt.so")
            mod = types.ModuleType("antenv.axon_hooks")
            mod.get_axon_ntff_profile_hook = lambda: hook
            mod.set_axon_ntff_profile_hook = lambda h: None
            sys.modules["antenv.axon_hooks"] = mod
        except Exception:
            pass
    _bir_fix_installed = True


# ---------------------------------------------------------------------------
# device kernel builder
# ---------------------------------------------------------------------------

_BUILD_CACHE = {}


def _chunks(cap, w=512):
    out = []
    off = 0
    while off < cap:
        out.append((off, min(w, cap - off)))
        off += w
    return out


def _build(caps, b1_nonzero, b2_nonzero, gb_nontrivial):
    import os

    mm_dtype = os.environ.get("MM_DTYPE", "f32r")
    key = (tuple(caps), b1_nonzero, b2_nonzero, gb_nontrivial, mm_dtype)
    if key in _BUILD_CACHE:
        return _BUILD_CACHE[key]

    import concourse.bass as bass
    import concourse.tile as tile
    from concourse import mybir

    f32 = mybir.dt.float32

    fmm = mybir.dt.float32r if mm_dtype == "f32r" else f32
    S = sum(caps)
    KD = D_MODEL // 128  # 2 chunks of the model dim
    KF = D_FF // 128  # 8 chunks of the ff dim

    nc = bass.Bass("TRN2")
    xg = nc.dram_tensor("xg", [KD, 128, S], fmm, kind="ExternalInput")
    w1t = nc.dram_tensor("w1t", [N_B, KD, 128, D_FF], fmm, kind="ExternalInput")
    D2 = D_MODEL + 2
    w2t = nc.dram_tensor("w2t", [N_B, KF, 128, D2], fmm, kind="ExternalInput")
    if b2_nonzero:
        b2e = nc.dram_tensor("b2e", [N_B, D2], f32, kind="ExternalInput")
    if b1_nonzero:
        b1d = nc.dram_tensor("b1", [N_B, D_FF], f32, kind="ExternalInput")
    if gb_nontrivial:
        gamd = nc.dram_tensor("gamma", [N_B, D_MODEL], f32, kind="ExternalInput")
        betd = nc.dram_tensor("beta", [N_B, D_MODEL], f32, kind="ExternalInput")
    yc = nc.dram_tensor("yc", [S, D_MODEL], f32, kind="ExternalOutput")

    AF = mybir.ActivationFunctionType
    OP = mybir.AluOpType

    with tile.TileContext(nc) as tc:
        with (
            tc.tile_pool(name="singles", bufs=1) as singles,
            tc.tile_pool(name="w1p", bufs=2) as w1p,
            tc.tile_pool(name="w2p", bufs=2) as w2p,
            tc.tile_pool(name="cns", bufs=2) as cns,
            tc.tile_pool(name="xp", bufs=3) as xp,
            tc.tile_pool(name="hp", bufs=2) as hp,
            tc.tile_pool(name="ep", bufs=3) as ep,
            tc.tile_pool(name="up", bufs=3) as up,
            tc.tile_pool(name="op_", bufs=8) as op_,
            tc.tile_pool(name="stp", bufs=8) as stp,
            tc.tile_pool(name="php", bufs=3, space="PSUM") as php,
            tc.tile_pool(name="pyp", bufs=2, space="PSUM") as pyp,
        ):
            ones_col = singles.tile([1, 128], f32)
            nc.vector.memset(ones_col, 1.0)
            eps_tile = singles.tile([128, 1], f32)
            nc.vector.memset(eps_tile, LN_EPS)
            if b1_nonzero:
                ones_row = singles.tile([1, 512], f32)
                nc.vector.memset(ones_row, 1.0)

            seg_off = 0
            for n in range(N_B):
                cap = caps[n]
                if cap == 0:
                    continue
                w1_sb = w1p.tile([128, KD, D_FF], fmm, tag="w1")
                for k in range(KD):
                    nc.sync.dma_start(out=w1_sb[:, k, :], in_=w1t[n, k])
                w2_sb = w2p.tile([128, KF, D2], fmm, tag="w2")
                if b2_nonzero:
                    b2e_sb = cns.tile([1, D2], f32, tag="b2e")
                if b1_nonzero:
                    b1_sb = cns.tile([1, D_FF], f32, tag="b1")
                    nc.sync.dma_start(out=b1_sb, in_=b1d[n : n + 1, :])
                if gb_nontrivial:
                    gam_bc = cns.tile([128, D_MODEL], f32, tag="gam")
                    bet_bc = cns.tile([128, D_MODEL], f32, tag="bet")
                    gsrc = gamd[n : n + 1, :]
                    bsrc = betd[n : n + 1, :]
                    nc.gpsimd.dma_start(
                        out=gam_bc,
                        in_=bass.AP(
                            tensor=gsrc.tensor,
                            offset=gsrc.offset,
                            ap=[[0, 128], gsrc.ap[1]],
                        ),
                    )
                    nc.gpsimd.dma_start(
                        out=bet_bc,
                        in_=bass.AP(
                            tensor=bsrc.tensor,
                            offset=bsrc.offset,
                            ap=[[0, 128], bsrc.ap[1]],
                        ),
                    )

                NT = cap // 128
                varall = stp.tile([128, NT], f32, tag="mv")
                rst = stp.tile([128, NT], f32, tag="rst")
                ycs = []
                for off, W in _chunks(cap):
                    goff = seg_off + off
                    nW = W // 128
                    xg_sb = xp.tile([128, KD, 512], fmm, tag="xg")
                    for k in range(KD):
                        nc.sync.dma_start(
                            out=xg_sb[:, k, :W], in_=xg[k, :, goff : goff + W]
                        )
                    if off == 0:
                        nc.sync.dma_start(
                            out=w2_sb, in_=w2t[n].rearrange("j p d -> p j d")
                        )
                        if b2_nonzero:
                            nc.sync.dma_start(out=b2e_sb, in_=b2e[n : n + 1, :])
                    h_sb = hp.tile([128, KF, 512], fmm, tag="h")
                    for fp in range(KF // 2):
                        ph = php.tile([128, 2, 512], f32, tag="ph")
                        for fi in range(2):
                            f = fp * 2 + fi
                            fs = slice(f * 128, (f + 1) * 128)
                            nc.tensor.matmul(
                                ph[:, fi, :W],
                                w1_sb[:, 0, fs],
                                xg_sb[:, 0, :W],
                                start=True,
                                stop=False,
                            )
                            nc.tensor.matmul(
                                ph[:, fi, :W],
                                w1_sb[:, 1, fs],
                                xg_sb[:, 1, :W],
                                start=False,
                                stop=not b1_nonzero,
                            )
                            if b1_nonzero:
                                nc.tensor.matmul(
                                    ph[:, fi, :W],
                                    b1_sb[:, fs],
                                    ones_row[:, :W],
                                    start=False,
                                    stop=True,
                                )
                        # elu(v) = relu(v) - relu(1 - exp(v))
                        e_sb = ep.tile([128, 2, 512], f32, tag="e")
                        nc.scalar.activation(e_sb[:, :, :W], ph[:, :, :W], AF.Exp)
                        u_sb = up.tile([128, 2, 512], f32, tag="u")
                        if fp < 4:
                            # u = relu(1 - E) on ACT (same table set as exp)
                            nc.scalar.activation(
                                u_sb[:, :, :W],
                                e_sb[:, :, :W],
                                AF.Relu,
                                bias=1.0,
                                scale=-1.0,
                            )
                            op1 = OP.subtract
                        else:
                            # v = (E min 1) - 1 = -u on DVE; STT adds it
                            nc.vector.tensor_scalar(
                                u_sb[:, :, :W],
                                e_sb[:, :, :W],
                                scalar1=1.0,
                                scalar2=1.0,
                                op0=OP.min,
                                op1=OP.subtract,
                            )
                            op1 = OP.add
                        nc.vector.scalar_tensor_tensor(
                            h_sb[:, fp * 2 : fp * 2 + 2, :W],
                            ph[:, :, :W],
                            0.0,
                            u_sb[:, :, :W],
                            op0=OP.max,
                            op1=op1,
                        )
                    yc_sb = op_.tile([128, 4, D_MODEL], f32, tag="o")
                    ycs.append((yc_sb, goff, W))
                    for t in range(nW):
                        tt = off // 128 + t
                        py = pyp.tile([128, D2], f32, tag="py")
                        for f in range(KF):
                            nc.tensor.matmul(
                                py,
                                h_sb[:, f, t * 128 : (t + 1) * 128],
                                w2_sb[:, f, :],
                                start=(f == 0),
                                stop=not b2_nonzero and f == KF - 1,
                            )
                        if b2_nonzero:
                            nc.tensor.matmul(
                                py,
                                ones_col[:, :128],
                                b2e_sb,
                                start=False,
                                stop=True,
                            )
                        # py[:, 256] = mean(y) (colsum(W2)/256 folded into w2t).
                        # centered y; scale by rstd later (batched per branch)
                        nc.vector.tensor_scalar_sub(
                            yc_sb[:, t, :],
                            py[:, :D_MODEL],
                            scalar1=py[:, D_MODEL : D_MODEL + 1],
                        )
                        # var = mean((y-mu)^2) via squared sum accumulator
                        vscr = stp.tile([128, D_MODEL], f32, tag="vscr")
                        nc.vector.scalar_tensor_tensor(
                            vscr,
                            yc_sb[:, t, :],
                            0.0,
                            yc_sb[:, t, :],
                            op0=OP.bypass,
                            op1=OP.mult,
                            accum_out=varall[:, tt : tt + 1],
                        )
                # rstd + apply + store. Batched per branch (one sqrt table
                # swap); for the last branch, per chunk-group so the tail
                # pipeline drains during compute.
                def flush(tiles, lo, hi):
                    sd = stp.tile([128, NT], f32, tag="sd")
                    nc.scalar.activation(
                        sd[:, : hi - lo],
                        varall[:, lo:hi],
                        AF.Sqrt,
                        bias=eps_tile,
                        scale=1.0 / D_MODEL,
                    )
                    nc.vector.reciprocal(rst[:, lo:hi], sd[:, : hi - lo])
                    for yc_sb, goff, W in tiles:
                        nW = W // 128
                        o2 = op_.tile([128, 4, D_MODEL], f32, tag="o2")
                        for t in range(nW):
                            tt = (goff - seg_off) // 128 + t
                            nc.vector.tensor_scalar_mul(
                                o2[:, t, :],
                                yc_sb[:, t, :],
                                scalar1=rst[:, tt : tt + 1],
                            )
                            if gb_nontrivial:
                                nc.vector.tensor_mul(
                                    o2[:, t, :], o2[:, t, :], gam_bc
                                )
                                nc.vector.tensor_add(
                                    o2[:, t, :], o2[:, t, :], bet_bc
                                )
                        nc.sync.dma_start(
                            out=yc[goff : goff + W, :].rearrange(
                                "(c p) d -> p c d", p=128
                            ),
                            in_=o2[:, :nW, :],
                        )

                last = n == max(i for i in range(N_B) if caps[i] > 0)
                if last:
                    half = (len(ycs) + 1) // 2
                    for grp in (ycs[:half], ycs[half:]):
                        if not grp:
                            continue
                        lo = (grp[0][1] - seg_off) // 128
                        hi = (grp[-1][1] - seg_off) // 128 + grp[-1][2] // 128
                        flush(grp, lo, hi)
                else:
                    lowprio = tc.high_priority(offset=-1000000)
                    lowprio.__enter__()
                    flush(ycs, 0, NT)
                    lowprio.__exit__(None, None, None)
                seg_off += cap

    _BUILD_CACHE[key] = nc
    return nc


# ---------------------------------------------------------------------------
# host wrapper
# ---------------------------------------------------------------------------


def kernel(x, b_seq, w1, b1, w2, b2, gamma, beta):
    _install_bir_fix()
    from concourse.bass_utils import run_bass_kernel_spmd

    x = np.asarray(x, dtype=np.float32)
    b_seq = np.asarray(b_seq, dtype=np.int32)
    w1 = np.asarray(w1, dtype=np.float32)
    b1 = np.asarray(b1, dtype=np.float32)
    w2 = np.asarray(w2, dtype=np.float32)
    b2 = np.asarray(b2, dtype=np.float32)
    gamma = np.asarray(gamma, dtype=np.float32)
    beta = np.asarray(beta, dtype=np.float32)

    x_flat = x.reshape(NTOK, D_MODEL)
    bs = b_seq.reshape(NTOK)

    # token ids per branch, split evenly over cores
    parts = []  # parts[n][c] -> int array of token ids
    for n in range(1, N_B + 1):
        idx = np.nonzero(bs == n)[0].astype(np.int64)
        parts.append(np.array_split(idx, NCORES))
    caps = []
    for n in range(N_B):
        mx = max(len(p) for p in parts[n])
        caps.append(0 if mx == 0 else ((mx + 127) // 128) * 128)
    S = sum(caps)

    b1_nonzero = bool(np.any(b1))
    b2_nonzero = bool(np.any(b2))
    gb_nontrivial = bool(np.any(beta)) or not bool(np.all(gamma == 1.0))

    nc = _build(tuple(caps), b1_nonzero, b2_nonzero, gb_nontrivial)

    # weight layouts
    w1t = np.ascontiguousarray(
        w1.transpose(0, 2, 1).reshape(N_B, D_MODEL // 128, 128, D_FF)
    )
    w2t_core = w2.transpose(0, 2, 1).reshape(N_B, D_FF // 128, 128, D_MODEL)
    w2_colmean = (w2.sum(axis=1) / D_MODEL).reshape(N_B, D_FF // 128, 128, 1)
    w2t = np.ascontiguousarray(
        np.concatenate(
            [w2t_core, w2_colmean, np.zeros_like(w2_colmean)], axis=3
        ),
        dtype=np.float32,
    )
    b2e = np.ascontiguousarray(
        np.concatenate(
            [b2, b2.sum(axis=1, keepdims=True) / D_MODEL, np.zeros((N_B, 1), np.float32)],
            axis=1,
        ),
        dtype=np.float32,
    )

    in_maps = []
    gidx_per_core = []
    for c in range(NCORES):
        gidx = np.zeros(S, dtype=np.int64)
        seg = 0
        for n in range(N_B):
            p = parts[n][c]
            gidx[seg : seg + len(p)] = p
            seg += caps[n]
        gidx_per_core.append(gidx)
        xgc = np.ascontiguousarray(x_flat[gidx].T.reshape(D_MODEL // 128, 128, S))
        m = {"xg": xgc, "w1t": w1t, "w2t": w2t}
        if b2_nonzero:
            m["b2e"] = b2e
        if b1_nonzero:
            m["b1"] = b1
        if gb_nontrivial:
            m["gamma"] = gamma
            m["beta"] = beta
        in_maps.append(m)

    import os
    import time

    trace = bool(os.environ.get("KERNEL_TRACE"))
    res = None
    for attempt in range(3):
        try:
            res = run_bass_kernel_spmd(
                nc, in_maps, core_ids=list(range(NCORES)), trace=trace
            )
            break
        except Exception:
            # transient NRT device errors have been observed on the first
            # execution of a freshly compiled NEFF; retry
            if attempt == 2:
                raise
            time.sleep(3)
    global LAST_RESULTS
    LAST_RESULTS = res

    out_flat = np.zeros((NTOK, D_MODEL), dtype=np.float32)
    for c in range(NCORES):
        ycc = res.results[c]["yc"]
        seg = 0
        for n in range(N_B):
            p = parts[n][c]
            out_flat[p] = ycc[seg : seg + len(p)]
            seg += caps[n]
    return out_flat.reshape(B, T, D_MODEL)
